# revision 1
# baseline (speedup 1.0000x reference)
"""Trainium2 Bass kernel for BinaryMLP:
    h = relu(x @ sign(w1).T + b1); h = relu(h @ sign(w2).T + b2);
    h = relu(h @ sign(w3).T + b3); y = h @ w4.T + b4

Data-parallel over 8 NeuronCores: batch 65536 -> 8192 rows/core, weights
replicated. On-device dataflow is feature-major ("transposed"): activations
live in SBUF as [feature_partition, batch_free] so every layer's contraction
dim (the feature/hidden dim) is the PE partition dim. The host only slices
the batch, transposes for layout, and concatenates the result back.

Compute is bf16 on the tensor engine (binary +-1 weights are exact in bf16;
PSUM accumulates fp32). Binarization (sign) and the fp32->bf16 casts happen
on-device on the vector engine; bias+relu runs on the scalar engine reading
PSUM and writing bf16 back to SBUF.
"""

import numpy as np

N_CORES = 8
F_IN = 784  # input features: 7 k-tiles of 112
K1 = 112
NK1 = 7
H = 512  # hidden width: 4 k-tiles / m-tiles of 128
NKH = 4
N_OUT = 10
CHUNK = 512  # batch columns per moving-operand chunk


def build_nc(b_shard: int, num_devices: int = N_CORES, chunk: int = CHUNK):
    """Build + compile the per-core Bass program for a batch shard of
    b_shard columns. Every core runs the identical program."""
    import concourse.bacc as bacc
    import concourse.mybir as mybir
    import concourse.tile as tile

    f32 = mybir.dt.float32
    bf16 = mybir.dt.bfloat16
    AluOp = mybir.AluOpType
    ActFn = mybir.ActivationFunctionType

    n_chunks = b_shard // chunk
    assert n_chunks * chunk == b_shard

    nc = bacc.Bacc(
        "TRN2", target_bir_lowering=False, debug=False, num_devices=num_devices
    )

    xT = nc.dram_tensor("xT", [F_IN, b_shard], f32, kind="ExternalInput")
    w1T = nc.dram_tensor("w1T", [F_IN, H], f32, kind="ExternalInput")
    w2T = nc.dram_tensor("w2T", [H, H], f32, kind="ExternalInput")
    w3T = nc.dram_tensor("w3T", [H, H], f32, kind="ExternalInput")
    w4T = nc.dram_tensor("w4T", [H, N_OUT], f32, kind="ExternalInput")
    b1 = nc.dram_tensor("b1", [H, 1], f32, kind="ExternalInput")
    b2 = nc.dram_tensor("b2", [H, 1], f32, kind="ExternalInput")
    b3 = nc.dram_tensor("b3", [H, 1], f32, kind="ExternalInput")
    b4 = nc.dram_tensor("b4", [N_OUT, 1], f32, kind="ExternalInput")
    y = nc.dram_tensor("y", [N_OUT, b_shard], f32, kind="ExternalOutput")

    with tile.TileContext(nc) as tc:
        with (
            tc.tile_pool(name="wconst", bufs=1) as wpool,
            tc.tile_pool(name="wstage", bufs=2) as wstage,
            tc.tile_pool(name="xin", bufs=4) as xin_pool,
            tc.tile_pool(name="xbf", bufs=10) as xbf_pool,
            tc.tile_pool(name="hbuf", bufs=8) as h_pool,
            tc.tile_pool(name="yout", bufs=3) as y_pool,
            tc.tile_pool(name="psum", bufs=8, space="PSUM") as ps_pool,
        ):
            # ---- weights: DMA latent fp32, binarize to +-1 bf16 on DVE ----
            def prep_bin(w_dram, n_k, k_size, name):
                tiles = []
                for k in range(n_k):
                    wf = wstage.tile([k_size, H], f32, tag="wstage", name=f"{name}f{k}")
                    nc.sync.dma_start(wf[:], w_dram.ap()[k * k_size : (k + 1) * k_size, :])
                    wb = wpool.tile([k_size, H], bf16, tag=f"{name}{k}", name=f"{name}{k}")
                    # wb = (wf >= 0) -> 1.0/0.0 ; then wb*2-1 -> +-1
                    nc.vector.tensor_scalar(wb[:], wf[:], 0.0, None, AluOp.is_ge)
                    nc.vector.tensor_scalar(
                        wb[:], wb[:], 2.0, -1.0, AluOp.mult, AluOp.add
                    )
                    tiles.append(wb)
                return tiles

            w1b = prep_bin(w1T, NK1, K1, "w1b")
            w2b = prep_bin(w2T, NKH, 128, "w2b")
            w3b = prep_bin(w3T, NKH, 128, "w3b")

            w4c = []
            for k in range(NKH):
                w4f = wstage.tile([128, N_OUT], f32, tag="w4stage", name=f"w4f{k}")
                nc.sync.dma_start(w4f[:], w4T.ap()[k * 128 : (k + 1) * 128, :])
                wc = wpool.tile([128, N_OUT], bf16, tag=f"w4c{k}", name=f"w4c{k}")
                nc.vector.tensor_copy(wc[:], w4f[:])
                w4c.append(wc)

            # ---- biases: per-partition scalars ----
            def load_bias(b_dram, n_k, k_size, name):
                tiles = []
                for k in range(n_k):
                    bt = wpool.tile([k_size, 1], f32, tag=f"{name}{k}", name=f"{name}{k}")
                    nc.sync.dma_start(bt[:], b_dram.ap()[k * k_size : (k + 1) * k_size, :])
                    tiles.append(bt)
                return tiles

            b1t = load_bias(b1, NKH, 128, "b1t")
            b2t = load_bias(b2, NKH, 128, "b2t")
            b3t = load_bias(b3, NKH, 128, "b3t")
            b4t = load_bias(b4, 1, N_OUT, "b4t")[0]

            # ---- main loop over batch chunks ----
            for c in range(n_chunks):
                csl = slice(c * chunk, (c + 1) * chunk)

                # load + cast x chunk (feature-major)
                xb = []
                for k in range(NK1):
                    xf = xin_pool.tile([K1, chunk], f32, tag="xf", name=f"xf_{c}_{k}")
                    nc.sync.dma_start(xf[:], xT.ap()[k * K1 : (k + 1) * K1, csl])
                    xk = xbf_pool.tile([K1, chunk], bf16, tag="xb", name=f"xb_{c}_{k}")
                    nc.vector.tensor_copy(xk[:], xf[:])
                    xb.append(xk)

                def layer(inputs, wtiles, btiles, n_k, name):
                    outs = []
                    for m in range(NKH):
                        ps = ps_pool.tile([128, chunk], f32, tag="ps", name=f"ps_{name}_{c}_{m}")
                        for k in range(n_k):
                            nc.tensor.matmul(
                                ps[:],
                                lhsT=wtiles[k][:, m * 128 : (m + 1) * 128],
                                rhs=inputs[k][:],
                                start=(k == 0),
                                stop=(k == n_k - 1),
                            )
                        ht = h_pool.tile([128, chunk], bf16, tag=f"h{name}", name=f"h{name}_{c}_{m}")
                        nc.scalar.activation(
                            ht[:], ps[:], ActFn.Relu, bias=btiles[m][:], scale=1.0
                        )
                        outs.append(ht)
                    return outs

                h1 = layer(xb, w1b, b1t, NK1, "1")
                h2 = layer(h1, w2b, b2t, NKH, "2")
                h3 = layer(h2, w3b, b3t, NKH, "3")

                # fc4: [10, chunk] = sum_k w4c[k].T @ h3[k]
                ps4 = ps_pool.tile([N_OUT, chunk], f32, tag="ps", name=f"ps4_{c}")
                for k in range(NKH):
                    nc.tensor.matmul(
                        ps4[:],
                        lhsT=w4c[k][:],
                        rhs=h3[k][:],
                        start=(k == 0),
                        stop=(k == NKH - 1),
                    )
                yt = y_pool.tile([N_OUT, chunk], f32, tag="yt", name=f"yt_{c}")
                nc.scalar.activation(
                    yt[:], ps4[:], ActFn.Identity, bias=b4t[:], scale=1.0
                )
                nc.sync.dma_start(y.ap()[:, csl], yt[:])

    nc.compile()
    return nc


_CACHE = {}


def _get_nc(b_shard: int):
    key = b_shard
    if key not in _CACHE:
        _CACHE[key] = build_nc(b_shard)
    return _CACHE[key]


def kernel(x, w1, b1, w2, b2, w3, b3, w4, b4):
    from concourse.bass_utils import run_bass_kernel_spmd

    B = x.shape[0]
    b_shard = B // N_CORES
    nc = _get_nc(b_shard)

    # host-side layout prep (no model math: slicing/transposes only)
    xT = np.ascontiguousarray(np.asarray(x, dtype=np.float32).T)
    w1T = np.ascontiguousarray(np.asarray(w1, dtype=np.float32).T)
    w2T = np.ascontiguousarray(np.asarray(w2, dtype=np.float32).T)
    w3T = np.ascontiguousarray(np.asarray(w3, dtype=np.float32).T)
    w4T = np.ascontiguousarray(np.asarray(w4, dtype=np.float32).T)
    common = {
        "w1T": w1T,
        "w2T": w2T,
        "w3T": w3T,
        "w4T": w4T,
        "b1": np.asarray(b1, np.float32).reshape(H, 1),
        "b2": np.asarray(b2, np.float32).reshape(H, 1),
        "b3": np.asarray(b3, np.float32).reshape(H, 1),
        "b4": np.asarray(b4, np.float32).reshape(N_OUT, 1),
    }
    in_maps = [
        {"xT": np.ascontiguousarray(xT[:, i * b_shard : (i + 1) * b_shard]), **common}
        for i in range(N_CORES)
    ]

    res = run_bass_kernel_spmd(nc, in_maps, core_ids=list(range(N_CORES)))
    yT = np.concatenate([res.results[i]["y"] for i in range(N_CORES)], axis=1)
    return np.ascontiguousarray(yT.T).astype(np.float32)
